# revision 58
# baseline (speedup 1.0000x reference)
"""Euclidean-distance attention on 8 Trainium2 NeuronCores.

Sharding: batch (2) x head-groups (4 heads each) -> 8 cores; each core
computes Q/K/V projections for its 4 heads (column-sliced weights), a
flash-style transposed-score attention, and a partial out-projection
(row-sliced wo). Host sums the 4 partials per batch (row-parallel out_proj
reduction) and adds the output bias.

Math trick: softmax_k(-max(||q||^2+||k||^2-2qk, 0)/T) == softmax_k((2qk-||k||^2)/T)
(the ||q||^2 term is constant per row and cancels; the max() clamp never fires
because d^2 >= 0 up to rounding).  With scores computed transposed
(scT[k, q] = K @ Q^T), the per-k bias -||k||^2/T is a per-partition vector and
folds into the scalar-engine exp activation: p~ = exp(scale*scT + bias).
Normalization uses an extra all-ones column appended to V, so the softmax
denominator falls out of the same PSUM accumulation as the numerator.

Key optimizations (362us baseline -> here):
- x arrives HOST-TRANSPOSED as bf16 [E, S]; weights host-cast to bf16.
  No on-chip transposes or casts; first matmul ~15us in (DMA-bw bound).
- reciprocal_approx_fast for softmax denominators (~51 ULP; the iterative
  reciprocal cost 53.6us of DVE).
- unnormalized attn output kept bf16 (denominator-broadcast matmuls at
  bf16 rate; fp32 matmuls are 4 cycles/row).
- -||k||^2/T comes from squaring the K^T projection PSUM on the scalar
  engine (bf16, scaled by 1/sqrt(T)) and a per-pair ones-matmul that
  reduces over the d-partitions -- this halves the V projection chain
  (wv alone instead of wv||wk), saving ~33K PE cycles.
- exp tiles are FD=1024 (2 q-blocks per ACTIVATE): per-call overhead is
  ~310ns, so FD=512 exps cost 1.6ns/elem vs 1.04 (measured; an FD=512
  variant regressed the whole kernel to 424us).
- wv/wo loads ride the gpsimd SWDGE queue: the two HWDGE queues carry
  only wq/wk + the x^T slabs the first projection chains wait on.
- the first three (h=0) score/exp pairs are pre-rolled at the end of
  phase 1 (closes a 5us exp-stream hole at the phase boundary); the
  interleaved out-projection tail is popped as two 2-matmul pieces
  sharing one PSUM allocation, so a pop never injects more PE work
  than two iterations of exp slack; y-store DMA triggers all issue
  from the sync engine (a dma_start costs the issuing engine ~650ns
  and the scalar engine is the exp-critical one).

Measured dead ends (kept out): fp8e4 DoubleRow projections (weight-quant
error ~4% is common-mode across attended tokens -> fails the 2e-2 gate);
DoubleRow 2x32 packing for the d=64 score contraction (1.8x SLOWER);
packed per-head FD=512 attention units (exp overhead + sc->exp lockstep,
424us); merging attention into the projection stream (the firmware power
throttler that halves the PE clock for ~100us follows total PE activity,
not scheduling -- moving work around just moves the cold window).
"""

import sys

sys.path.insert(0, "/opt/trn_rl_repo")

import numpy as np

import concourse.bass as bass
import concourse.tile as tile
from concourse import bacc, mybir
from concourse.bass_utils import run_bass_kernel_spmd

F32 = mybir.dt.float32
BF16 = mybir.dt.bfloat16

E = 1024          # embed dim
D = 64            # head dim
HLOC = 4          # heads per core
DH = HLOC * D     # 256: per-core projection width
P = 128
N_CORES = 8


def build_program(S, temperature, zq, zk, zv):
    """Trace the per-core program. All 8 cores run this same program on
    different input slices. zq/zk/zv: bias-is-zero flags (skip the adds)."""
    T = float(temperature)
    NT = S // P           # token tiles (16)
    NE = E // P           # embed (contraction) tiles (8)
    NPR = HLOC // 2       # head pairs (2)
    QW = min(512, S)      # q block width for score matmuls
    NQB = S // QW         # q blocks (4)
    GW = 2 * QW           # exp tile width (2 q-blocks share one ACT call)
    JB = QW // P          # token tiles per block (4)

    nc = bacc.Bacc(None)
    xT_d = nc.dram_tensor("xT", [E, S], BF16, kind="ExternalInput")
    wq_d = nc.dram_tensor("wq_s", [E, DH], BF16, kind="ExternalInput")
    wk_d = nc.dram_tensor("wk_s", [E, DH], BF16, kind="ExternalInput")
    wv_d = nc.dram_tensor("wv_s", [E, DH], BF16, kind="ExternalInput")
    wo_d = nc.dram_tensor("wo_s", [DH, E], BF16, kind="ExternalInput")
    bq_d = nc.dram_tensor("bq_s", [DH], F32, kind="ExternalInput")
    bk_d = nc.dram_tensor("bk_s", [DH], F32, kind="ExternalInput")
    bv_d = nc.dram_tensor("bv_s", [DH], F32, kind="ExternalInput")
    # one output tensor per token tile; bf16 partials summed on host in f32
    y_ds = [
        nc.dram_tensor(f"y{tt}", [P, E], BF16, kind="ExternalOutput")
        for tt in range(NT)
    ]

    def bcast_ap(ap_1d, parts):
        # [N] dram vector -> [parts, N] partition-broadcast AP
        return bass.AP(
            tensor=ap_1d.tensor, offset=ap_1d.offset, ap=[[0, parts]] + list(ap_1d.ap)
        )

    with tile.TileContext(nc) as tc:
        with tc.tile_pool(name="consts", bufs=1) as consts, \
             tc.tile_pool(name="big", bufs=1) as big, \
             tc.tile_pool(name="sqpool", bufs=3) as sqpool, \
             tc.tile_pool(name="pTpool", bufs=10) as pTpool, \
             tc.tile_pool(name="dbpool", bufs=4) as dbpool, \
             tc.tile_pool(name="ypool", bufs=4) as ypool:
            # ---- constants / weights staging ----
            wq_sb = consts.tile([P, NE, DH], BF16)
            wk_sb = consts.tile([P, NE, DH], BF16)
            wv_sb = consts.tile([P, NE, DH], BF16)
            wo_sb = consts.tile([P, 2, E], BF16)

            # all-ones stationary for the denominator broadcast matmul;
            # row 64 (= base_partition of the denominator row) is what's used
            ones_col = consts.tile([P, D], BF16)
            nc.vector.memset(ones_col, 1.0)
            # [-1 on rows 0-63 | -1 on rows 64-127] columns: reduces the
            # squared K^T over each head's d-partitions via one matmul
            negones2 = consts.tile([P, 2], BF16)
            nc.vector.memset(negones2, 0.0)
            nc.vector.memset(negones2[0:D, 0:1], -1.0)
            nc.vector.memset(negones2[D:P, 1:2], -1.0)

            if not (zq and zk):
                bq_col = consts.tile([P, NPR], F32)
                nc.gpsimd.dma_start(bq_col, bq_d[:].rearrange("(pr p) -> p pr", p=P))
                bk_col = consts.tile([P, NPR], F32)
                nc.gpsimd.dma_start(bk_col, bk_d[:].rearrange("(pr p) -> p pr", p=P))
            else:
                bq_col = bk_col = None

            if not zv:
                bv_bc = consts.tile([P, DH], F32)
                nc.gpsimd.dma_start(bv_bc, bcast_ap(bv_d[:], P))

            # ---- persistent big tiles ----
            # x^T slabs, DMA'd directly from the host-transposed bf16 input:
            # qT[p, e, s] = x[s, e*128+p]
            qT = big.tile([P, NE, S], BF16)
            QT_sb = big.tile([P, NPR, S], BF16)      # Q^T per head-pair
            KT_sb = big.tile([P, NPR, S], BF16)
            V_sb = big.tile([P, NT, HLOC, D + 1], BF16)   # V + ones column
            nksq = big.tile([P, NT, HLOC], F32)      # -||k||^2 / T
            ou_all = big.tile([P, HLOC, S], BF16)    # unnormalized attn out
            aoT = big.tile([P, NPR, S], BF16)        # normalized attn out^T

            nc.gpsimd.memset(V_sb[:, :, :, D], 1.0)

            # Input DMAs spread over four queues: wq/wk + 6 x^T slabs on
            # the two HWDGE queues (the first projection chains block on
            # these), the last-consumed slabs e6/e7 on the gpsimd/vector
            # SWDGE queues, wv/wo behind them (not needed until
            # ~30us/~60us in).
            nc.sync.dma_start(
                wq_sb[:, :, :], wq_d[:].rearrange("(e p) d -> p e d", p=P))
            nc.scalar.dma_start(
                wk_sb[:, :, :], wk_d[:].rearrange("(e p) d -> p e d", p=P))
            for e in range(NE - 2):
                eng = nc.sync if e % 2 == 0 else nc.scalar
                eng.dma_start(qT[:, e, :], xT_d[e * P:(e + 1) * P, :])
            # last two slabs on the gpsimd SWDGE queue: slower per byte,
            # but it unloads the HWDGE queues and keeps early PE duty low
            # enough that the firmware power throttler stays off (moving
            # these back to HWDGE measured 309us vs 247us)
            nc.gpsimd.dma_start(qT[:, NE - 2, :],
                                xT_d[(NE - 2) * P:(NE - 1) * P, :])
            nc.gpsimd.dma_start(qT[:, NE - 1, :],
                                xT_d[(NE - 1) * P:NE * P, :])
            nc.gpsimd.dma_start(
                wv_sb[:, :, :], wv_d[:].rearrange("(e p) d -> p e d", p=P))
            nc.gpsimd.dma_start(
                wo_sb[:, :, :], wo_d[:].rearrange("(s p) d -> p s d", p=P))

            # ---- phase 1: projections ----
            sT = 1.0 / float(np.sqrt(T))
            # attention pipeline state shared across the phase boundary:
            # the first three (h=0, g0=0) score/exp pairs are emitted at
            # the end of phase 1 (inputs are all block-0 products) so the
            # scalar engine's exp stream has no hole when phase 2 starts
            # (measured 5us boundary gap)
            pending = []  # (g0, h, j, pT_t)

            def emit_sc_exp(g0, h, j, pool, tag):
                pr = h // 2
                off = (h % 2) * D
                sc_t = pool.tile([P, GW], F32, tag=tag, name=f"se{h}_{j}",
                                 bufs=1)
                for qq in range(2):
                    qb = g0 + qq
                    nc.tensor.matmul(
                        sc_t[:, qq * QW:(qq + 1) * QW],
                        lhsT=KT_sb[off:off + D, pr, j * P:(j + 1) * P],
                        rhs=QT_sb[off:off + D, pr, qb * QW:(qb + 1) * QW],
                        start=True,
                        stop=True,
                    )
                pT_t = pTpool.tile([P, GW], BF16, tag="pT")
                nc.scalar.activation(
                    out=pT_t,
                    in_=sc_t,
                    func=mybir.ActivationFunctionType.Exp,
                    bias=nksq[:, j, h:h + 1],
                    scale=2.0 / T,
                )
                pending.append((g0, h, j, pT_t))

            with tc.tile_pool(name="ps_pj", bufs=2, space="PSUM") as ps_pj, \
                 tc.tile_pool(name="ps_kv", bufs=3, space="PSUM") as ps_kv:
                for blk in range(NQB):
                    bsl = slice(blk * QW, (blk + 1) * QW)
                    jlo = blk * JB
                    # Q^T and K^T per head pair over this token block
                    for pr in range(NPR):
                        psl = slice(pr * P, (pr + 1) * P)
                        for qk, (dst, w_sb, bz) in enumerate(
                                ((QT_sb, wq_sb, zq), (KT_sb, wk_sb, zk))):
                            pj = ps_pj.tile([P, QW], F32, tag="pj")
                            for e in range(NE):
                                nc.tensor.matmul(
                                    pj,
                                    lhsT=w_sb[:, e, psl],
                                    rhs=qT[:, e, bsl],
                                    start=(e == 0),
                                    stop=(e == NE - 1),
                                )
                            if qk == 1:
                                # -||k||^2/T from the K^T psum: square on
                                # ACT (scaled by 1/sqrt(T), bf16), reduce
                                # over each head's 64 d-partitions with one
                                # ones-matmul per token tile
                                sq_t = sqpool.tile([P, QW], BF16, tag="sq")
                                if zk:
                                    sq_in = pj
                                else:
                                    # bias is per d-row: one column add
                                    kb_t = sqpool.tile([P, QW], F32,
                                                       tag="kb")
                                    nc.vector.tensor_scalar_add(
                                        out=kb_t, in0=pj,
                                        scalar1=bk_col[:, pr:pr + 1],
                                    )
                                    sq_in = kb_t
                                nc.scalar.activation(
                                    out=sq_t, in_=sq_in,
                                    func=mybir.ActivationFunctionType.Square,
                                    scale=sT,
                                )
                                nk = ps_kv.tile([P, 2 * DH], F32, tag="pv",
                                                name=f"nk{blk}_{pr}")
                                for c in range(JB):
                                    nc.tensor.matmul(
                                        nk[:, c * 2:(c + 1) * 2],
                                        lhsT=sq_t[:, c * P:(c + 1) * P],
                                        rhs=negones2,
                                        start=True,
                                        stop=True,
                                    )
                                nc.vector.tensor_copy(
                                    nksq[:, jlo:jlo + JB, 2 * pr:2 * pr + 2],
                                    nk[:, 0:2 * JB].rearrange(
                                        "p (c h) -> p c h", h=2),
                                )
                            if bz:
                                nc.vector.tensor_copy(dst[:, pr, bsl], pj)
                            else:
                                bcol = bq_col if qk == 0 else bk_col
                                nc.vector.tensor_scalar_add(
                                    out=dst[:, pr, bsl], in0=pj,
                                    scalar1=bcol[:, pr:pr + 1],
                                )
                    # V (token-major) over this token block
                    for j in range(jlo, jlo + JB):
                        pvk = ps_kv.tile([P, 2 * DH], F32, tag="pv")
                        for e in range(NE):
                            nc.tensor.matmul(
                                pvk[:, 0:DH],
                                lhsT=qT[:, e, j * P:(j + 1) * P],
                                rhs=wv_sb[:, e, :],
                                start=(e == 0),
                                stop=(e == NE - 1),
                            )
                        vdst = V_sb[:, j, :, 0:D]
                        pvr = pvk[:, 0:DH].rearrange("p (h d) -> p h d",
                                                     h=HLOC)
                        if zv:
                            nc.vector.tensor_copy(vdst, pvr)
                        else:
                            nc.vector.tensor_add(
                                out=vdst, in0=pvr,
                                in1=bv_bc.rearrange("p (h d) -> p h d", h=HLOC),
                            )
                        # pre-roll (h=0, g0=0) score/exp pairs j=0..7 under
                        # the V-chains of the following blocks: moves ~9us
                        # of exp work into this ACT-idle PE-bound region
                        # (their attn*V debt is drained at a rate the exp
                        # slack can absorb, see the j%2 pop below)
                        if blk >= 1:
                            pj_j = (blk - 1) * JB + (j - jlo)
                            if pj_j < 8:
                                emit_sc_exp(0, 0, pj_j, ps_pj, "se")



            # ---- phase 2: attention, software-pipelined ----
            # Per (q-block-pair, head): the score matmul for token tile j+1
            # is emitted BEFORE the attn*V matmul of tile j, so the tensor
            # engine streams scores while the scalar engine runs exp.  The
            # previous pair's normalization + out-projection is interleaved
            # one task per 4 j-iterations so no engine drains at pair
            # boundaries.
            with tc.tile_pool(name="ps_sc", bufs=2, space="PSUM") as ps_sc, \
                 tc.tile_pool(name="ps_av", bufs=1, space="PSUM") as ps_av, \
                 tc.tile_pool(name="ps_tl", bufs=1, space="PSUM") as ps_tl:

                def norm_task(g0, h, pool, tag):
                    # normalize head h for BOTH q-blocks of pair g0 at once
                    def run():
                        pr = h // 2
                        off = (h % 2) * D
                        csl = slice(g0 * QW, (g0 + 2) * QW)
                        bct = pool.tile([P, GW], F32, tag=tag,
                                        name=f"bc{g0}_{h}")
                        # matmul output is capped at 512 fp32 (one PSUM
                        # bank) per instruction, so broadcast per q-block
                        for qq in range(2):
                            qsl = slice((g0 + qq) * QW, (g0 + qq + 1) * QW)
                            nc.tensor.matmul(
                                bct[:D, qq * QW:(qq + 1) * QW],
                                lhsT=ones_col[D:D + 1, :],
                                rhs=ou_all[D:D + 1, h, qsl],
                                start=True,
                                stop=True,
                            )
                        rb = dbpool.tile([D, GW], F32, tag="rb")
                        nc.vector.reciprocal_approx_fast(rb, bct[:D, :])
                        nc.vector.tensor_mul(
                            aoT[off:off + D, pr, csl],
                            ou_all[:D, h, csl],
                            rb,
                        )
                    return run

                def oj_task(tt, pool, tag, act_copy=False):
                    def run(pool=pool, tag=tag):
                        py = pool.tile([P, GW], F32, tag=tag, name=f"py{tt}")
                        for oh in range(E // QW):
                            for s in range(2):
                                nc.tensor.matmul(
                                    py[:, oh * QW:(oh + 1) * QW],
                                    lhsT=aoT[:, s, tt * P:(tt + 1) * P],
                                    rhs=wo_sb[:, s, oh * QW:(oh + 1) * QW],
                                    start=(s == 0),
                                    stop=(s == 1),
                                )
                        yt = ypool.tile([P, E], BF16, tag="y")
                        if act_copy:
                            # tail flush: exp stream is over, the scalar
                            # engine is idle while DVE is the bottleneck
                            nc.scalar.activation(
                                out=yt, in_=py,
                                func=mybir.ActivationFunctionType.Copy)
                        else:
                            nc.vector.tensor_copy(yt, py)
                        nc.sync.dma_start(y_ds[tt][:, :], yt)
                    run.needs_pool = pool is None
                    return run

                def oj_parts(tt, pool, tag):
                    # the 4 out-proj matmuls split across FOUR tail pops
                    # (~213ns PE each, within one iteration of exp slack);
                    # all parts share one PSUM allocation
                    st = {}

                    def mk(oh, s, first, last):
                        def p():
                            if first:
                                st["py"] = pool.tile([P, GW], F32, tag=tag,
                                                     name=f"py{tt}")
                            py = st["py"]
                            nc.tensor.matmul(
                                py[:, oh * QW:(oh + 1) * QW],
                                lhsT=aoT[:, s, tt * P:(tt + 1) * P],
                                rhs=wo_sb[:, s, oh * QW:(oh + 1) * QW],
                                start=(s == 0),
                                stop=(s == 1),
                            )
                            if last:
                                yt = ypool.tile([P, E], BF16, tag="y")
                                nc.vector.tensor_copy(yt, py)
                                # store trigger on the sync engine only: a
                                # dma_start costs the issuing engine ~650ns
                                # and the scalar engine is exp-critical here
                                nc.sync.dma_start(y_ds[tt][:, :], yt)
                        return p

                    return [mk(oh, s, (oh, s) == (0, 0), (oh, s) == (1, 1))
                            for oh in range(2) for s in range(2)]

                def norm_parts(g0, h, pool, tag):
                    # the two denominator-broadcast matmuls as separate
                    # pops; the DVE recip+mul ride with the second
                    st = {}
                    pr = h // 2
                    off = (h % 2) * D
                    csl = slice(g0 * QW, (g0 + 2) * QW)

                    def p1():
                        st["bct"] = pool.tile([P, GW], F32, tag=tag,
                                              name=f"bc{g0}_{h}")
                        qsl = slice(g0 * QW, (g0 + 1) * QW)
                        nc.tensor.matmul(
                            st["bct"][:D, 0:QW],
                            lhsT=ones_col[D:D + 1, :],
                            rhs=ou_all[D:D + 1, h, qsl],
                            start=True,
                            stop=True,
                        )

                    def p2():
                        bct = st["bct"]
                        qsl = slice((g0 + 1) * QW, (g0 + 2) * QW)
                        nc.tensor.matmul(
                            bct[:D, QW:GW],
                            lhsT=ones_col[D:D + 1, :],
                            rhs=ou_all[D:D + 1, h, qsl],
                            start=True,
                            stop=True,
                        )
                        rb = dbpool.tile([D, GW], F32, tag="rb")
                        nc.vector.reciprocal_approx_fast(rb, bct[:D, :])
                        nc.vector.tensor_mul(
                            aoT[off:off + D, pr, csl],
                            ou_all[:D, h, csl],
                            rb,
                        )

                    return [p1, p2]

                tail = []
                av_cur = {}

                def emit_av(ent):
                    g0_, h_, j_, pT_ = ent
                    if j_ == 0:
                        # allocate the accumulator at emission time so the
                        # bufs=1 bank rotation sees the previous head's last
                        # writes/drain strictly before this head's reset
                        av_cur["t"] = ps_av.tile([P, GW], F32, tag="av",
                                                 name=f"av{g0_}_{h_}")
                    av_ = av_cur["t"]
                    for qq in range(2):
                        nc.tensor.matmul(
                            av_[:D + 1, qq * QW:(qq + 1) * QW],
                            lhsT=V_sb[:, j_, h_, :],
                            rhs=pT_[:, qq * QW:(qq + 1) * QW],
                            start=(j_ == 0),
                            stop=(j_ == NT - 1),
                        )
                    if j_ == NT - 1:
                        # drain unnormalized outputs; frees the av bank for
                        # the next head while normalization runs elsewhere
                        nc.vector.tensor_copy(
                            ou_all[:D + 1, h_, g0_ * QW:(g0_ + 2) * QW],
                            av_[:D + 1, :],
                        )
                        if g0_ + 2 >= NQB:
                            # final pair: normalize each head as soon as its
                            # accumulators drain (dedicated tail banks only:
                            # the sc/av banks are still live)
                            tail.extend(norm_parts(g0_, h_, ps_tl, "tl"))

                for g0 in range(0, NQB, 2):
                    last = (g0 + 2 >= NQB)
                    for h in range(HLOC):
                        pr = h // 2
                        off = (h % 2) * D
                        # (h=0, g0=0) j=0..7 were pre-rolled in phase 1
                        j0 = 8 if (g0 == 0 and h == 0) else 0
                        for j in range(j0, NT):
                            sc_t = ps_sc.tile([P, GW], F32, tag="sc")
                            for qq in range(2):
                                qb = g0 + qq
                                nc.tensor.matmul(
                                    sc_t[:, qq * QW:(qq + 1) * QW],
                                    lhsT=KT_sb[off:off + D, pr, j * P:(j + 1) * P],
                                    rhs=QT_sb[off:off + D, pr, qb * QW:(qb + 1) * QW],
                                    start=True,
                                    stop=True,
                                )
                            if len(pending) >= 3:
                                emit_av(pending.pop(0))
                            if len(pending) >= 5 and j % 2 == 0:
                                # drain the pre-roll backlog at +0.5 av
                                # per iteration (~213ns of extra PE per
                                # iter, inside the exp stream's slack)
                                emit_av(pending.pop(0))
                            pT_t = pTpool.tile([P, GW], BF16, tag="pT")
                            nc.scalar.activation(
                                out=pT_t,
                                in_=sc_t,
                                func=mybir.ActivationFunctionType.Exp,
                                bias=nksq[:, j, h:h + 1],
                                scale=2.0 / T,
                            )
                            pending.append((g0, h, j, pT_t))
                            if tail:
                                tail.pop(0)()
                    if not last:
                        # enqueue this pair's normalization + out-projection
                        # (as single-matmul pops), one per j-iteration of
                        # the next pair
                        for h in range(HLOC):
                            tail.extend(norm_parts(g0, h, ps_tl, "tl"))
                        for qq in range(2):
                            qb = g0 + qq
                            for tt in range(JB * qb, JB * qb + JB):
                                tail.extend(oj_parts(tt, ps_tl, "tl"))
                while pending:
                    emit_av(pending.pop(0))
                # final pair's out-projections go last: they need the h=3
                # normalizations that are only enqueued by the drain above
                for qb in range(NQB - 2, NQB):
                    for i, tt in enumerate(range(JB * qb, JB * qb + JB)):
                        tail.append(oj_task(tt, None, None, act_copy=True))
                # flush: attention banks are idle now, rotate through all
                # three PSUM pools for a deeper tail pipeline
                pts = [(ps_tl, "tl"), (ps_sc, "sc"), (ps_av, "av")]
                k = 0
                while tail:
                    t = tail.pop(0)
                    if getattr(t, "needs_pool", False):
                        pool, tag = pts[k % len(pts)]
                        k += 1
                        t(pool, tag)
                    else:
                        t()

    # run Bacc's compile passes (wait legalization, register allocation)
    nc.finalize()
    return nc


def make_in_maps(inputs, S):
    import ml_dtypes
    BF = ml_dtypes.bfloat16
    q = np.asarray(inputs["query"], np.float32)
    wq = np.asarray(inputs["wq"], np.float32).astype(BF)
    wk = np.asarray(inputs["wk"], np.float32).astype(BF)
    wv = np.asarray(inputs["wv"], np.float32).astype(BF)
    wo = np.asarray(inputs["wo"], np.float32).astype(BF)
    bq = np.asarray(inputs["bq"], np.float32)
    bk = np.asarray(inputs["bk"], np.float32)
    bv = np.asarray(inputs["bv"], np.float32)
    xT = [np.ascontiguousarray(q[b, :S].T).astype(BF) for b in range(q.shape[0])]
    in_maps = []
    for c in range(N_CORES):
        b = c // 4
        lo = (c % 4) * DH
        in_maps.append({
            "xT": xT[b],
            "wq_s": np.ascontiguousarray(wq[:, lo:lo + DH]),
            "wk_s": np.ascontiguousarray(wk[:, lo:lo + DH]),
            "wv_s": np.ascontiguousarray(wv[:, lo:lo + DH]),
            "wo_s": np.ascontiguousarray(wo[lo:lo + DH, :]),
            "bq_s": np.ascontiguousarray(bq[lo:lo + DH]),
            "bk_s": np.ascontiguousarray(bk[lo:lo + DH]),
            "bv_s": np.ascontiguousarray(bv[lo:lo + DH]),
        })
    return in_maps


_prog_cache = {}


def _get_program(S, T, zq, zk, zv):
    key = (S, T, zq, zk, zv)
    if key not in _prog_cache:
        _prog_cache[key] = build_program(S, T, zq, zk, zv)
    return _prog_cache[key]


def _run(inputs, trace=False, tmpdir=None):
    S = np.asarray(inputs["query"]).shape[1]
    T = float(np.asarray(inputs["temperature"]))
    zq = not np.any(np.asarray(inputs["bq"]))
    zk = not np.any(np.asarray(inputs["bk"]))
    zv = not np.any(np.asarray(inputs["bv"]))
    nc = _get_program(S, T, zq, zk, zv)
    in_maps = make_in_maps(inputs, S)
    res = run_bass_kernel_spmd(
        nc, in_maps, list(range(N_CORES)), trace=trace, tmpdir=tmpdir
    )
    ng = S // 128
    ys = [
        np.concatenate(
            [np.asarray(res.results[i][f"y{g}"]).astype(np.float32)
             for g in range(ng)],
            axis=0,
        )
        for i in range(N_CORES)
    ]
    bo = np.asarray(inputs["bo"], np.float32)
    out = np.stack([
        ys[0] + ys[1] + ys[2] + ys[3],
        ys[4] + ys[5] + ys[6] + ys[7],
    ]).astype(np.float32)
    out += bo[None, None, :]
    return out, res


def kernel(**inputs):
    out, _ = _run(inputs, trace=False)
    return out
